# revision 46
# baseline (speedup 1.0000x reference)
"""Trainium2 Bass kernel: causal multi-head attention with RoPE.

Problem: B=2, S=2048, D=1024, H=16 heads, hd=64, fp32 reference.
Sharding: 4-way head-tensor-parallel x 2-way batch-data-parallel over 8 cores.
Each core handles one batch element and 4 heads, computes its partial
contribution to the output projection; the host sums the 4 partials.

v2 design (all-bf16, PE-streaming-roofline oriented):
  - x transposed on HOST, sent bf16: no device DMA transposes, no hi/lo.
  - single Q/K projection; RoPE on device via a signed-permutation matmul:
    q_rot = p (.) cos + (P @ p) (.) sin  (interleaved pairing).
  - scores^T/exp/PV per (head-pair, q-chunk, k-block-pair) with causal
    diagonal trimming (partial-M matmuls + partial exp + [128,128] tri mask).
  - softmax denominator via ones-column in v (PSUM row 64); reciprocal on DVE
    (reciprocal_approx_fast), partition-broadcast on Pool, normalize on DVE.
  - y = otn.T @ wo accumulated over head pairs in PSUM, DMA'd straight from
    PSUM as f32 partials.
  - global software pipelining: attention rounds are interleaved with filler
    units (late projections, then y-projection) so the PE never waits on ACT.
"""
import numpy as np
import ml_dtypes
from collections import deque
from contextlib import ExitStack

import concourse.bass as bass
import concourse.tile as tile
from concourse import bacc, mybir
from concourse.bass_utils import run_bass_kernel_spmd

F32 = mybir.dt.float32
BF16 = mybir.dt.bfloat16

B, S, D, H, HD = 2, 2048, 1024, 16, 64
NCORES = 8
TPG = 4            # head-TP degree
LH = H // TPG      # 4 local heads
LD = LH * HD       # 256 local dims
ROPE_BASE = 10000.0
QC = 512           # q chunk
NQC = S // QC      # 4
NDT = D // 128     # 8

Exp = mybir.ActivationFunctionType.Exp

DEBUG = False
DEBUG2 = False   # end-of-kernel otn dump only
_NC_CACHE = None


def _build():
    nc = bacc.Bacc("TRN2", target_bir_lowering=False, debug=False,
                   enable_asserts=True, num_devices=NCORES)

    xt_d = nc.dram_tensor("xt", [NDT, 128, S], BF16, kind="ExternalInput").ap()
    wq_d = nc.dram_tensor("wq", [2, 128, 1024], BF16, kind="ExternalInput").ap()
    wk_d = nc.dram_tensor("wk", [2, 128, 1024], BF16, kind="ExternalInput").ap()
    wv_d = nc.dram_tensor("wv", [128, 2048], BF16, kind="ExternalInput").ap()
    wo_d = nc.dram_tensor("wo", [2, 128, D], BF16, kind="ExternalInput").ap()
    cos_d = nc.dram_tensor("cos", [128, S], BF16, kind="ExternalInput").ap()
    sin_d = nc.dram_tensor("sin", [128, S], BF16, kind="ExternalInput").ap()
    perm_d = nc.dram_tensor("perm", [128, 128], BF16, kind="ExternalInput").ap()
    pm_d = nc.dram_tensor("pm", [2, 128, 1024], BF16, kind="ExternalInput").ap()
    y_d = nc.dram_tensor("y", [S, D], BF16, kind="ExternalOutput").ap()
    if DEBUG2:
        otn2_d = nc.dram_tensor("otn2_dbg", [2, 128, S], BF16,
                                kind="ExternalOutput").ap()
        qr2_d = nc.dram_tensor("qr2_dbg", [2, 128, S], BF16,
                               kind="ExternalOutput").ap()
        kr2_d = nc.dram_tensor("kr2_dbg", [2, 128, S], BF16,
                               kind="ExternalOutput").ap()
        v2_d = nc.dram_tensor("v2_dbg", [16, 128, 260], BF16,
                              kind="ExternalOutput").ap()
        pt2_d = nc.dram_tensor("pt2_dbg", [6, 128, 1024], BF16,
                               kind="ExternalOutput").ap()
        po2_d = nc.dram_tensor("po2_dbg", [2, 128, 512], F32,
                               kind="ExternalOutput").ap()
        nr2_d = nc.dram_tensor("nr2_dbg", [2, 2, 512], F32,
                               kind="ExternalOutput").ap()
    if DEBUG:
        qr_d = nc.dram_tensor("qr_dbg", [2, 128, S], BF16,
                              kind="ExternalOutput").ap()
        kr_d = nc.dram_tensor("kr_dbg", [2, 128, S], BF16,
                              kind="ExternalOutput").ap()
        v_dbg = nc.dram_tensor("v_dbg", [4, 128, 260], BF16,
                               kind="ExternalOutput").ap()
        otn_d = nc.dram_tensor("otn_dbg", [2, 128, S], BF16,
                               kind="ExternalOutput").ap()
        den_d = nc.dram_tensor("den_dbg", [2, 3, 512], F32,
                               kind="ExternalOutput").ap()

    with tile.TileContext(nc) as tc, ExitStack() as octx:
        pers = octx.enter_context(tc.tile_pool(name="pers", bufs=1))

        # ---- input DMAs, spread across engine queues ----
        xt = [pers.tile([128, S], BF16, tag=f"xt{dt}", name=f"xt{dt}")
              for dt in range(NDT)]
        wq = pers.tile([128, 2048], BF16, tag="wq")     # [:, jt*1024+dt*128+c]
        wk = pers.tile([128, 2048], BF16, tag="wk")
        wv = pers.tile([128, 2048], BF16, tag="wv")     # [:, dt*256+c]
        wo = pers.tile([128, 2 * D], BF16, tag="wo")    # [:, hp*D+m]
        cos_sb = pers.tile([128, S], BF16, tag="cos")
        sin_sb = pers.tile([128, S], BF16, tag="sin")
        perm_sb = pers.tile([128, 128], BF16, tag="perm")
        pm_sb = [pers.tile([128, 1024], BF16, tag=f"pm{j}", name=f"pm{j}")
                 for j in range(2)]

        # 3 DMA-capable queues; x is split column-wise so the first M=512
        # projection chunk only waits on ~1.25MB, not the full 4MB of x.
        # Each destination tile is written from exactly ONE queue (per-queue
        # completion is in-order) to avoid multi-queue write-tracking races.
        dmaq = [nc.sync, nc.scalar, nc.gpsimd]
        nc.scalar.dma_start(xt[0][:, 0:512], xt_d[0][:, 0:512])
        nc.sync.dma_start(wq[:, 0:512], wq_d[0][:, 0:512])
        nc.gpsimd.dma_start(wq[:, 512:1024], wq_d[0][:, 512:1024])
        nc.scalar.dma_start(perm_sb[:], perm_d)
        nc.gpsimd.dma_start(wk[:, 0:1024], wk_d[0])
        for cs in range(4):
            c0 = cs * 512
            for dt in range(NDT):
                if dt == 0 and cs == 0:
                    continue
                dmaq[dt % 3].dma_start(xt[dt][:, c0:c0 + 512],
                                       xt_d[dt][:, c0:c0 + 512])
            dmaq[0].dma_start(cos_sb[:, c0:c0 + 512], cos_d[:, c0:c0 + 512])
            dmaq[1].dma_start(sin_sb[:, c0:c0 + 512], sin_d[:, c0:c0 + 512])
            if cs == 1:
                nc.scalar.dma_start(wv[:], wv_d)
                nc.gpsimd.dma_start(pm_sb[0][:], pm_d[0])
                nc.gpsimd.dma_start(pm_sb[1][:], pm_d[1])
            if cs == 2:
                # jt=1 weights early enough for the q1/k1 filler units
                nc.sync.dma_start(wq[:, 1024:2048], wq_d[1])
                nc.gpsimd.dma_start(wk[:, 1024:2048], wk_d[1])
        nc.gpsimd.dma_start(wo[:, 0:D], wo_d[0])
        nc.gpsimd.dma_start(wo[:, D:2 * D], wo_d[1])

        # ---- persistent compute tiles ----
        qrot = [pers.tile([128, S], BF16, tag=f"qrot{j}", name=f"qrot{j}")
                for j in range(2)]
        krot = [pers.tile([128, S], BF16, tag=f"krot{j}", name=f"krot{j}")
                for j in range(2)]
        # v natural + per-head ones column at col lh*65+64
        vsb = [pers.tile([128, 260], BF16, tag=f"v{st}", name=f"v{st}")
               for st in range(S // 128)]
        otn = [pers.tile([128, S], BF16, tag=f"otn{hp}", name=f"otn{hp}")
               for hp in range(2)]
        ones_f = pers.tile([128, 4], F32, tag="ones_f")
        nc.vector.memset(ones_f[:], 1.0)
        for st in range(S // 128):
            vdst = vsb[st].rearrange("p (h c) -> p h c", c=65)[:, :, 64:65]
            nc.vector.tensor_copy(vdst, ones_f[:].rearrange(
                "p (h c) -> p h c", c=1))

        # ---- PSUM pools (8 banks of [128,512]f32 equivalents) ----
        psp = octx.enter_context(tc.tile_pool(name="psp", bufs=2, space="PSUM"))
        pop = octx.enter_context(tc.tile_pool(name="pop", bufs=2, space="PSUM"))

        # ---- SBUF working pools ----
        wrk = octx.enter_context(tc.tile_pool(name="wrk", bufs=1))
        ptp = octx.enter_context(tc.tile_pool(name="ptp", bufs=1))

        rot_of = {0: qrot, 1: krot}
        w_of = {0: wq, 1: wk}

        def proj_mm(ppool, qk, jt, sc):
            """M=512 projection chunk: 8 accumulating matmuls + ACT copy."""
            c0 = sc * 512
            pp = ppool.tile([128, 512], F32, tag="pp", name="pp")
            for dt in range(NDT):
                nc.tensor.matmul(
                    pp[:], w_of[qk][:, jt * 1024 + dt * 128:
                                    jt * 1024 + dt * 128 + 128],
                    xt[dt][:, c0:c0 + 512],
                    start=(dt == 0), stop=(dt == NDT - 1))
            psb = wrk.tile([128, 512], BF16, tag="psb", bufs=4, name="psb")
            nc.scalar.copy(psb[:], pp[:])
            return psb

        def rope_mm(ppool, qk, jt, sc, psb):
            """perm matmul + rope combine for one chunk."""
            c0 = sc * 512
            pr = ppool.tile([128, 512], F32, tag="pp", name="pr")
            nc.tensor.matmul(pr[:], perm_sb[:], psb[:], start=True, stop=True)
            t1 = wrk.tile([128, 512], BF16, tag="t1", bufs=3, name="t1")
            nc.vector.tensor_mul(t1[:], psb[:], cos_sb[:, c0:c0 + 512])
            t2 = wrk.tile([128, 512], BF16, tag="t2", bufs=3, name="t2")
            nc.vector.tensor_mul(t2[:], pr[:], sin_sb[:, c0:c0 + 512])
            nc.vector.tensor_add(rot_of[qk][jt][:, c0:c0 + 512], t1[:], t2[:])

        def proj_unit(ppool, qk, jt, sc):
            rope_mm(ppool, qk, jt, sc, proj_mm(ppool, qk, jt, sc))

        def v_unit(ppool, st):
            """v projection for one 128-row s tile: 8 matmuls M=256."""
            pv = ppool.tile([128, 512], F32, tag="pp", name="pv")
            for dt in range(NDT):
                nc.tensor.matmul(pv[:, 0:256],
                                 xt[dt][:, st * 128:(st + 1) * 128],
                                 wv[:, dt * 256:(dt + 1) * 256],
                                 start=(dt == 0), stop=(dt == NDT - 1))
            dst = vsb[st].rearrange("p (h c) -> p h c", c=65)[:, :, 0:64]
            src = pv[:, 0:256].rearrange("p (h c) -> p h c", c=64)
            nc.vector.tensor_copy(dst, src)

        fillers = deque()

        def pump(n=1):
            for _ in range(n):
                if fillers:
                    fillers.popleft()()

        if DEBUG2:
            posnap = [pers.tile([128, 512], F32, tag=f"posnap{z}",
                                name=f"posnap{z}") for z in range(2)]
            rdsnap = [pers.tile([1, 512], F32, tag=f"rdsnap{z}",
                                name=f"rdsnap{z}") for z in range(2)]
            pbsnap = [pers.tile([1, 512], F32, tag=f"pbsnap{z}",
                                name=f"pbsnap{z}") for z in range(2)]

        def norm_chain(hp, qc, po):
            """po -> recip(DVE) -> bcast(Pool) -> otn(DVE, bf16)."""
            c0 = qc * QC
            for z in range(2):
                den_s = wrk.tile([1, QC], F32, tag="den_s", bufs=4,
                                 name="den_s")
                nc.vector.tensor_copy(den_s[:], po[z][64:65, :])
                rden = wrk.tile([1, QC], F32, tag="rden", bufs=4, name="rden")
                nc.vector.reciprocal_approx_fast(rden[:], den_s[:])
                pb = wrk.tile([64, QC], F32, tag="pb", bufs=4, name="pb")
                nc.gpsimd.partition_broadcast(pb[:], rden[:])
                nc.vector.tensor_mul(otn[hp][64 * z:64 * z + 64, c0:c0 + QC],
                                     po[z][0:64, :], pb[:])
                if DEBUG2 and hp == 0 and qc == 2:
                    nc.vector.tensor_copy(posnap[z][0:65, :], po[z][0:65, :])
                    nc.vector.tensor_copy(rdsnap[z][:], rden[:])
                    nc.vector.tensor_copy(pbsnap[z][:], pb[0:1, :])
                if DEBUG and hp == 0 and qc == 0:
                    den_sb = pers.tile([1, 512], F32, tag=f"dend{z}")
                    nc.vector.tensor_copy(den_sb[:], po[z][64:65, :])
                    nc.sync.dma_start(den_d[z, 0:1, :], den_sb[:])
                    nc.sync.dma_start(den_d[z, 1:2, :], rden[:])
                    nc.sync.dma_start(den_d[z, 2:3, :], pb[0:1, :])

        # persistent pt ring, memset once so stale regions are never NaN
        pt_ring = [pers.tile([128, 1024], BF16, tag=f"ptr{i}", name=f"ptr{i}")
                   for i in range(6)]
        for t in pt_ring:
            nc.vector.memset(t[:], 0.0)
        pt_ctr = [0]

        def attention(hp, ypool=None, boundary_pump=0):
            for qc in range(NQC):
                npair = 2 * qc + 2
                c0 = qc * QC
                last_kb = 4 * qc + 3
                # fillers run only between qc's, while no PSUM accumulation
                # chain (po) is open on the PE: a third concurrently-open
                # chain corrupts accumulation state.
                pump(boundary_pump)
                po = [pop.tile([128, 512], F32, tag="po", name=f"po{z}")
                      for z in range(2)]

                def emit_pv(kp, pts, po=po, hp=hp, last_kb=last_kb):
                    kb0 = 2 * kp
                    for z in range(2):
                        lh = 2 * hp + z
                        for e in range(2):
                            kb = kb0 + e
                            nc.tensor.matmul(
                                po[z][0:65, :],
                                vsb[kb][:, lh * 65:lh * 65 + 65],
                                pts[z][:, e * 512:(e + 1) * 512],
                                start=(kb == 0), stop=(kb == last_kb))

                pend = deque()
                for kp in range(npair):
                    kb0 = 2 * kp
                    diag = kp - (npair - 2)
                    pts = []
                    for z in range(2):
                        r0 = 64 * z
                        ps_ = psp.tile([128, 1024], F32, tag="ps", name="ps_")
                        pt = pt_ring[pt_ctr[0] % 6]
                        pt_ctr[0] += 1
                        for e in range(2):
                            kb = kb0 + e
                            m0 = 128 * (2 * diag + e) if diag >= 0 else 0
                            nc.tensor.matmul(
                                ps_[:, e * 512 + m0:(e + 1) * 512],
                                krot[hp][r0:r0 + 64, kb * 128:(kb + 1) * 128],
                                qrot[hp][r0:r0 + 64, c0 + m0:c0 + QC],
                                start=True, stop=True, tile_position=(r0, 0))
                        if diag < 0:
                            nc.scalar.activation(pt[:], ps_[:], Exp,
                                                 scale=0.125)
                        else:
                            for e in range(2):
                                m0 = 128 * (2 * diag + e)
                                nc.scalar.activation(
                                    pt[:, e * 512 + m0:(e + 1) * 512],
                                    ps_[:, e * 512 + m0:(e + 1) * 512],
                                    Exp, scale=0.125)
                            # full-tile mask: tri on diag blocks, zeros on
                            # uncomputed regions, ones elsewhere
                            nc.vector.tensor_mul(pt[:], pt[:],
                                                 pm_sb[diag][:])
                        pts.append(pt)
                    pend.append((kp, pts))
                    if len(pend) > 2:
                        emit_pv(*pend.popleft())
                if qc == 0:
                    # safe pump point: scores emitted, po chain not yet open
                    pump(4)
                while pend:
                    emit_pv(*pend.popleft())
                norm_chain(hp, qc, po)
                if ypool is not None:
                    for st in range(4 * qc, 4 * qc + 4):
                        fillers.append(lambda st=st: y_unit(ypool, st))

        def y_unit(ypool, st):
            """output projection for one 128-row s tile; bf16 stage + DMA."""
            ysb = wrk.tile([128, D], BF16, tag="ysb", bufs=3, name="ysb")
            for mc in range(2):
                py = ypool.tile([128, 512], F32, tag="py", name="py")
                for hp in range(2):
                    nc.tensor.matmul(py[:],
                                     otn[hp][:, st * 128:(st + 1) * 128],
                                     wo[:, hp * D + mc * 512:
                                        hp * D + (mc + 1) * 512],
                                     start=(hp == 0), stop=(hp == 1))
                dst = ysb[:, mc * 512:(mc + 1) * 512]
                if mc == 0:
                    nc.scalar.copy(dst, py[:])
                else:
                    nc.vector.tensor_copy(dst, py[:])
            dmaq[st % 3].dma_start(y_d[st * 128:(st + 1) * 128, :], ysb[:])

        # ---- emission ----
        with ExitStack() as s1:
            pp1 = s1.enter_context(tc.tile_pool(name="pp1", bufs=2,
                                                space="PSUM"))
            # early projections (head-pair 0), lag-1 pipelined
            punits = [(0, 0, sc) for sc in range(4)] + \
                     [(1, 0, sc) for sc in range(4)]
            prev = None
            for u in punits:
                psb = proj_mm(pp1, *u)
                if prev is not None:
                    rope_mm(pp1, *prev[0], prev[1])
                prev = (u, psb)
            rope_mm(pp1, *prev[0], prev[1])
            for st in range(4):
                v_unit(pp1, st)
            # fillers for attention(0): v tiles 4..15, q/k head-pair 1
            # v units first: their inputs (xt, wv) are resident early, so
            # they can fill the qc0/qc1 PE gaps; q1/k1 units need the late
            # jt=1 weight DMAs and are pumped at later boundaries.
            for st in range(4, 16):
                fillers.append(lambda st=st: v_unit(pp1, st))
            for qk in range(2):
                for sc in range(4):
                    fillers.append(
                        lambda qk=qk, sc=sc: proj_unit(pp1, qk, 1, sc))
            attention(0, boundary_pump=6)
            while fillers:
                pump()
        with ExitStack() as s2:
            pyp = s2.enter_context(tc.tile_pool(name="pyp", bufs=2,
                                                space="PSUM"))
            attention(1, ypool=pyp, boundary_pump=5)
            while fillers:
                pump()
        if DEBUG2:
            for j in range(2):
                nc.sync.dma_start(otn2_d[j], otn[j][:])
                nc.sync.dma_start(qr2_d[j], qrot[j][:])
                nc.sync.dma_start(kr2_d[j], krot[j][:])
            for st in range(16):
                nc.gpsimd.dma_start(v2_d[st], vsb[st][:])
            for i in range(6):
                nc.scalar.dma_start(pt2_d[i], pt_ring[i][:])
            for z in range(2):
                nc.sync.dma_start(po2_d[z], posnap[z][:])
                nc.sync.dma_start(nr2_d[z, 0:1, :], rdsnap[z][:])
                nc.sync.dma_start(nr2_d[z, 1:2, :], pbsnap[z][:])
        if DEBUG:
            for j in range(2):
                nc.sync.dma_start(qr_d[j], qrot[j][:])
                nc.sync.dma_start(kr_d[j], krot[j][:])
                nc.sync.dma_start(otn_d[j], otn[j][:])
            for st in range(4):
                nc.gpsimd.dma_start(v_dbg[st], vsb[st][:])

    nc.compile()
    return nc


def _get_nc():
    global _NC_CACHE
    if _NC_CACHE is None:
        _NC_CACHE = _build()
    return _NC_CACHE


def _host_prep(x, Wq, Wk, Wv, Wo):
    """Build the 8 per-core input maps."""
    bf = ml_dtypes.bfloat16
    x = np.asarray(x, dtype=np.float32)
    Wq, Wk, Wv, Wo = (np.asarray(w, dtype=np.float32) for w in (Wq, Wk, Wv, Wo))

    # rope tables: rows r = z*64 + d, angle index (r % 64) // 2
    t = np.arange(32, dtype=np.float64)
    theta = 1.0 / (ROPE_BASE ** (2.0 * t / HD))
    ang = np.arange(S, dtype=np.float64)[:, None] * theta[None, :]  # [S, 32]
    c32 = np.cos(ang).T.astype(np.float32)        # [32, S]
    s32 = np.sin(ang).T.astype(np.float32)
    cexp = np.repeat(c32, 2, axis=0)              # [64, S] rows 2i,2i+1 = c_i
    sexp = np.repeat(s32, 2, axis=0)
    cos_t = np.ascontiguousarray(np.tile(cexp, (2, 1))).astype(bf)   # [128, S]
    sin_t = np.ascontiguousarray(np.tile(sexp, (2, 1))).astype(bf)

    # signed permutation: out[2i] = -p[2i+1], out[2i+1] = p[2i]
    P = np.zeros((128, 128), dtype=np.float32)
    i2 = np.arange(0, 128, 2)
    P[i2 + 1, i2] = -1.0
    P[i2, i2 + 1] = 1.0
    P = np.ascontiguousarray(P.astype(bf))

    # pair masks pm[j][p, e*512+u] = 1 iff p <= u - 128*(2j+e): covers the
    # per-kb triangle, zeroes the uncomputed (trimmed) score regions, ones
    # elsewhere.
    p_ = np.arange(128)[:, None]
    u_ = np.arange(512)[None, :]
    pm = np.zeros((2, 128, 1024), dtype=np.float32)
    for j in range(2):
        for e in range(2):
            pm[j][:, e * 512:(e + 1) * 512] = (p_ <= u_ - 128 * (2 * j + e))
    pm = np.ascontiguousarray(pm.astype(bf))

    per_b = []
    for b in range(B):
        xtb = np.ascontiguousarray(x[b].T).astype(bf)   # [D, S]
        per_b.append(np.ascontiguousarray(xtb.reshape(NDT, 128, S)))

    def qk_tiles(W, g):
        A = W[g * LD:(g + 1) * LD, :]                   # [256, 1024]
        A4 = A.reshape(2, 128, NDT, 128)                # [jt, c, dt, p]
        return np.ascontiguousarray(
            A4.transpose(0, 3, 2, 1).reshape(2, 128, 1024).astype(bf))

    per_g = []
    for g in range(TPG):
        wq_t = qk_tiles(Wq, g)
        wk_t = qk_tiles(Wk, g)
        V = Wv[g * LD:(g + 1) * LD, :]                  # [256, 1024]
        V3 = V.reshape(256, NDT, 128)                   # [c, dt, p]
        wv_t = np.ascontiguousarray(
            V3.transpose(2, 1, 0).reshape(128, 2048).astype(bf))
        Wo_g = Wo[:, g * LD:(g + 1) * LD]               # [1024, 256]
        wo_t = np.ascontiguousarray(Wo_g.T.reshape(2, 128, D).astype(bf))
        per_g.append((wq_t, wk_t, wv_t, wo_t))

    in_maps = []
    for c in range(NCORES):
        b, g = divmod(c, TPG)
        wq_t, wk_t, wv_t, wo_t = per_g[g]
        in_maps.append({
            "xt": per_b[b],
            "wq": wq_t, "wk": wk_t, "wv": wv_t, "wo": wo_t,
            "cos": cos_t, "sin": sin_t, "perm": P, "pm": pm,
        })
    return in_maps


def run(inputs, trace=False):
    """Run on all 8 cores; returns (y_full, BassKernelResults)."""
    in_maps = _host_prep(inputs["x"], inputs["Wq"], inputs["Wk"],
                         inputs["Wv"], inputs["Wo"])
    nc = _get_nc()
    kw = {}
    if trace:
        kw = dict(trace=True, trace_cores=[0])
    res = run_bass_kernel_spmd(nc, in_maps, core_ids=list(range(NCORES)), **kw)
    y = np.zeros((B, S, D), dtype=np.float32)
    for c in range(NCORES):
        y[c // TPG] += np.asarray(res.results[c]["y"], dtype=np.float32)
    return y, res


def kernel(x, Wq, Wk, Wv, Wo, n_heads):
    assert int(n_heads) == H
    y, _ = run({"x": x, "Wq": Wq, "Wk": Wk, "Wv": Wv, "Wo": Wo})
    return y


# revision 48
# speedup vs baseline: 1.0438x; 1.0438x over previous
"""Trainium2 Bass kernel: causal multi-head attention with RoPE.

Problem: B=2, S=2048, D=1024, H=16 heads, hd=64, fp32 reference.
Sharding: 4-way head-tensor-parallel x 2-way batch-data-parallel over 8 cores.
Each core handles one batch element and 4 heads, computes its partial
contribution to the output projection; the host sums the 4 partials.

v2 design (all-bf16, PE-streaming-roofline oriented):
  - x transposed on HOST, sent bf16: no device DMA transposes, no hi/lo.
  - single Q/K projection; RoPE on device via a signed-permutation matmul:
    q_rot = p (.) cos + (P @ p) (.) sin  (interleaved pairing).
  - scores^T/exp/PV per (head-pair, q-chunk, k-block-pair) with causal
    diagonal trimming (partial-M matmuls + partial exp + [128,128] tri mask).
  - softmax denominator via ones-column in v (PSUM row 64); reciprocal on DVE
    (reciprocal_approx_fast), partition-broadcast on Pool, normalize on DVE.
  - y = otn.T @ wo accumulated over head pairs in PSUM, DMA'd straight from
    PSUM as f32 partials.
  - global software pipelining: attention rounds are interleaved with filler
    units (late projections, then y-projection) so the PE never waits on ACT.
"""
import numpy as np
import ml_dtypes
from collections import deque
from contextlib import ExitStack

import concourse.bass as bass
import concourse.tile as tile
from concourse import bacc, mybir
from concourse.bass_utils import run_bass_kernel_spmd

F32 = mybir.dt.float32
BF16 = mybir.dt.bfloat16

B, S, D, H, HD = 2, 2048, 1024, 16, 64
NCORES = 8
TPG = 4            # head-TP degree
LH = H // TPG      # 4 local heads
LD = LH * HD       # 256 local dims
ROPE_BASE = 10000.0
QC = 512           # q chunk
NQC = S // QC      # 4
NDT = D // 128     # 8

Exp = mybir.ActivationFunctionType.Exp

DEBUG = False
DEBUG2 = False   # end-of-kernel otn dump only
_NC_CACHE = None


def _build():
    nc = bacc.Bacc("TRN2", target_bir_lowering=False, debug=False,
                   enable_asserts=True, num_devices=NCORES)

    xt_d = nc.dram_tensor("xt", [NDT, 128, S], BF16, kind="ExternalInput").ap()
    wq_d = nc.dram_tensor("wq", [2, 128, 1024], BF16, kind="ExternalInput").ap()
    wk_d = nc.dram_tensor("wk", [2, 128, 1024], BF16, kind="ExternalInput").ap()
    wv_d = nc.dram_tensor("wv", [128, 2048], BF16, kind="ExternalInput").ap()
    wo_d = nc.dram_tensor("wo", [2, 128, D], BF16, kind="ExternalInput").ap()
    cos_d = nc.dram_tensor("cos", [128, S], BF16, kind="ExternalInput").ap()
    sin_d = nc.dram_tensor("sin", [128, S], BF16, kind="ExternalInput").ap()
    perm_d = nc.dram_tensor("perm", [128, 128], BF16, kind="ExternalInput").ap()
    pm_d = nc.dram_tensor("pm", [2, 128, 1024], BF16, kind="ExternalInput").ap()
    y_d = nc.dram_tensor("y", [S, D], BF16, kind="ExternalOutput").ap()
    if DEBUG2:
        otn2_d = nc.dram_tensor("otn2_dbg", [2, 128, S], BF16,
                                kind="ExternalOutput").ap()
        qr2_d = nc.dram_tensor("qr2_dbg", [2, 128, S], BF16,
                               kind="ExternalOutput").ap()
        kr2_d = nc.dram_tensor("kr2_dbg", [2, 128, S], BF16,
                               kind="ExternalOutput").ap()
        v2_d = nc.dram_tensor("v2_dbg", [16, 128, 260], BF16,
                              kind="ExternalOutput").ap()
        pt2_d = nc.dram_tensor("pt2_dbg", [6, 128, 1024], BF16,
                               kind="ExternalOutput").ap()
        po2_d = nc.dram_tensor("po2_dbg", [2, 128, 512], F32,
                               kind="ExternalOutput").ap()
        nr2_d = nc.dram_tensor("nr2_dbg", [2, 2, 512], F32,
                               kind="ExternalOutput").ap()
    if DEBUG:
        qr_d = nc.dram_tensor("qr_dbg", [2, 128, S], BF16,
                              kind="ExternalOutput").ap()
        kr_d = nc.dram_tensor("kr_dbg", [2, 128, S], BF16,
                              kind="ExternalOutput").ap()
        v_dbg = nc.dram_tensor("v_dbg", [4, 128, 260], BF16,
                               kind="ExternalOutput").ap()
        otn_d = nc.dram_tensor("otn_dbg", [2, 128, S], BF16,
                               kind="ExternalOutput").ap()
        den_d = nc.dram_tensor("den_dbg", [2, 3, 512], F32,
                               kind="ExternalOutput").ap()

    with tile.TileContext(nc) as tc, ExitStack() as octx:
        pers = octx.enter_context(tc.tile_pool(name="pers", bufs=1))

        # ---- input DMAs, spread across engine queues ----
        xt = [pers.tile([128, S], BF16, tag=f"xt{dt}", name=f"xt{dt}")
              for dt in range(NDT)]
        wq = pers.tile([128, 2048], BF16, tag="wq")     # [:, jt*1024+dt*128+c]
        wk = pers.tile([128, 2048], BF16, tag="wk")
        wv = pers.tile([128, 2048], BF16, tag="wv")     # [:, dt*256+c]
        wo = pers.tile([128, 2 * D], BF16, tag="wo")    # [:, hp*D+m]
        cos_sb = pers.tile([128, S], BF16, tag="cos")
        sin_sb = pers.tile([128, S], BF16, tag="sin")
        perm_sb = pers.tile([128, 128], BF16, tag="perm")
        pm_sb = [pers.tile([128, 1024], BF16, tag=f"pm{j}", name=f"pm{j}")
                 for j in range(2)]

        # 3 DMA-capable queues; x is split column-wise so the first M=512
        # projection chunk only waits on ~1.25MB, not the full 4MB of x.
        # Each destination tile is written from exactly ONE queue (per-queue
        # completion is in-order) to avoid multi-queue write-tracking races.
        dmaq = [nc.sync, nc.scalar, nc.gpsimd]
        nc.scalar.dma_start(xt[0][:, 0:512], xt_d[0][:, 0:512])
        nc.sync.dma_start(wq[:, 0:512], wq_d[0][:, 0:512])
        nc.gpsimd.dma_start(wq[:, 512:1024], wq_d[0][:, 512:1024])
        nc.scalar.dma_start(perm_sb[:], perm_d)
        nc.gpsimd.dma_start(wk[:, 0:1024], wk_d[0])
        for cs in range(4):
            c0 = cs * 512
            for dt in range(NDT):
                if dt == 0 and cs == 0:
                    continue
                dmaq[dt % 3].dma_start(xt[dt][:, c0:c0 + 512],
                                       xt_d[dt][:, c0:c0 + 512])
            dmaq[0].dma_start(cos_sb[:, c0:c0 + 512], cos_d[:, c0:c0 + 512])
            dmaq[1].dma_start(sin_sb[:, c0:c0 + 512], sin_d[:, c0:c0 + 512])
            if cs == 1:
                nc.scalar.dma_start(wv[:], wv_d)
                nc.gpsimd.dma_start(pm_sb[0][:], pm_d[0])
                nc.gpsimd.dma_start(pm_sb[1][:], pm_d[1])
            if cs == 2:
                # jt=1 weights early enough for the q1/k1 filler units
                nc.sync.dma_start(wq[:, 1024:2048], wq_d[1])
                nc.gpsimd.dma_start(wk[:, 1024:2048], wk_d[1])
        nc.gpsimd.dma_start(wo[:, 0:D], wo_d[0])
        nc.gpsimd.dma_start(wo[:, D:2 * D], wo_d[1])

        # ---- persistent compute tiles ----
        qrot = [pers.tile([128, S], BF16, tag=f"qrot{j}", name=f"qrot{j}")
                for j in range(2)]
        krot = [pers.tile([128, S], BF16, tag=f"krot{j}", name=f"krot{j}")
                for j in range(2)]
        # v natural + per-head ones column at col lh*65+64
        vsb = [pers.tile([128, 260], BF16, tag=f"v{st}", name=f"v{st}")
               for st in range(S // 128)]
        otn = [pers.tile([128, S], BF16, tag=f"otn{hp}", name=f"otn{hp}")
               for hp in range(2)]
        ones_f = pers.tile([128, 4], F32, tag="ones_f")
        nc.vector.memset(ones_f[:], 1.0)
        for st in range(S // 128):
            vdst = vsb[st].rearrange("p (h c) -> p h c", c=65)[:, :, 64:65]
            nc.vector.tensor_copy(vdst, ones_f[:].rearrange(
                "p (h c) -> p h c", c=1))

        # ---- PSUM pools (8 banks of [128,512]f32 equivalents) ----
        psp = octx.enter_context(tc.tile_pool(name="psp", bufs=2, space="PSUM"))
        pop = octx.enter_context(tc.tile_pool(name="pop", bufs=2, space="PSUM"))

        # ---- SBUF working pools ----
        wrk = octx.enter_context(tc.tile_pool(name="wrk", bufs=1))
        ptp = octx.enter_context(tc.tile_pool(name="ptp", bufs=1))

        rot_of = {0: qrot, 1: krot}
        w_of = {0: wq, 1: wk}

        def proj_mm(ppool, qk, jt, sc):
            """M=512 projection chunk: 8 accumulating matmuls + ACT copy."""
            c0 = sc * 512
            pp = ppool.tile([128, 512], F32, tag="pp", name="pp")
            for dt in range(NDT):
                nc.tensor.matmul(
                    pp[:], w_of[qk][:, jt * 1024 + dt * 128:
                                    jt * 1024 + dt * 128 + 128],
                    xt[dt][:, c0:c0 + 512],
                    start=(dt == 0), stop=(dt == NDT - 1))
            psb = wrk.tile([128, 512], BF16, tag="psb", bufs=4, name="psb")
            nc.scalar.copy(psb[:], pp[:])
            return psb

        def rope_mm(ppool, qk, jt, sc, psb):
            """perm matmul + rope combine for one chunk."""
            c0 = sc * 512
            pr = ppool.tile([128, 512], F32, tag="pp", name="pr")
            nc.tensor.matmul(pr[:], perm_sb[:], psb[:], start=True, stop=True)
            t1 = wrk.tile([128, 512], BF16, tag="t1", bufs=3, name="t1")
            nc.vector.tensor_mul(t1[:], psb[:], cos_sb[:, c0:c0 + 512])
            t2 = wrk.tile([128, 512], BF16, tag="t2", bufs=3, name="t2")
            nc.vector.tensor_mul(t2[:], pr[:], sin_sb[:, c0:c0 + 512])
            nc.vector.tensor_add(rot_of[qk][jt][:, c0:c0 + 512], t1[:], t2[:])

        def proj_unit(ppool, qk, jt, sc):
            rope_mm(ppool, qk, jt, sc, proj_mm(ppool, qk, jt, sc))

        def v_unit(ppool, st):
            """v projection for one 128-row s tile: 8 matmuls M=256."""
            pv = ppool.tile([128, 512], F32, tag="pp", name="pv")
            for dt in range(NDT):
                nc.tensor.matmul(pv[:, 0:256],
                                 xt[dt][:, st * 128:(st + 1) * 128],
                                 wv[:, dt * 256:(dt + 1) * 256],
                                 start=(dt == 0), stop=(dt == NDT - 1))
            dst = vsb[st].rearrange("p (h c) -> p h c", c=65)[:, :, 0:64]
            src = pv[:, 0:256].rearrange("p (h c) -> p h c", c=64)
            nc.vector.tensor_copy(dst, src)

        fillers = deque()

        def pump(n=1):
            for _ in range(n):
                if fillers:
                    fillers.popleft()()

        if DEBUG2:
            posnap = [pers.tile([128, 512], F32, tag=f"posnap{z}",
                                name=f"posnap{z}") for z in range(2)]
            rdsnap = [pers.tile([1, 512], F32, tag=f"rdsnap{z}",
                                name=f"rdsnap{z}") for z in range(2)]
            pbsnap = [pers.tile([1, 512], F32, tag=f"pbsnap{z}",
                                name=f"pbsnap{z}") for z in range(2)]

        def norm_chain(hp, qc, po):
            """po -> recip(DVE) -> bcast(Pool) -> otn(DVE, bf16)."""
            c0 = qc * QC
            for z in range(2):
                den_s = wrk.tile([1, QC], F32, tag="den_s", bufs=4,
                                 name="den_s")
                nc.vector.tensor_copy(den_s[:], po[z][64:65, :])
                rden = wrk.tile([1, QC], F32, tag="rden", bufs=4, name="rden")
                nc.vector.reciprocal_approx_fast(rden[:], den_s[:])
                pb = wrk.tile([64, QC], F32, tag="pb", bufs=4, name="pb")
                nc.gpsimd.partition_broadcast(pb[:], rden[:])
                nc.vector.tensor_mul(otn[hp][64 * z:64 * z + 64, c0:c0 + QC],
                                     po[z][0:64, :], pb[:])
                if DEBUG2 and hp == 0 and qc == 2:
                    nc.vector.tensor_copy(posnap[z][0:65, :], po[z][0:65, :])
                    nc.vector.tensor_copy(rdsnap[z][:], rden[:])
                    nc.vector.tensor_copy(pbsnap[z][:], pb[0:1, :])
                if DEBUG and hp == 0 and qc == 0:
                    den_sb = pers.tile([1, 512], F32, tag=f"dend{z}")
                    nc.vector.tensor_copy(den_sb[:], po[z][64:65, :])
                    nc.sync.dma_start(den_d[z, 0:1, :], den_sb[:])
                    nc.sync.dma_start(den_d[z, 1:2, :], rden[:])
                    nc.sync.dma_start(den_d[z, 2:3, :], pb[0:1, :])

        # persistent pt ring, memset once so stale regions are never NaN
        pt_ring = [pers.tile([128, 1024], BF16, tag=f"ptr{i}", name=f"ptr{i}")
                   for i in range(6)]
        for t in pt_ring:
            nc.vector.memset(t[:], 0.0)
        pt_ctr = [0]

        def attention(hp, ypool=None, boundary_pump=0):
            for qc in range(NQC):
                npair = 2 * qc + 2
                c0 = qc * QC
                last_kb = 4 * qc + 3
                # fillers run only between qc's, while no PSUM accumulation
                # chain (po) is open on the PE: a third concurrently-open
                # chain corrupts accumulation state. qc0 starts immediately
                # (early fillers would stall on in-flight input DMAs).
                pump(boundary_pump if qc > 0 else 0)
                po = [pop.tile([128, 512], F32, tag="po", name=f"po{z}")
                      for z in range(2)]

                def emit_pv(kp, pts, po=po, hp=hp, last_kb=last_kb):
                    kb0 = 2 * kp
                    for z in range(2):
                        lh = 2 * hp + z
                        for e in range(2):
                            kb = kb0 + e
                            nc.tensor.matmul(
                                po[z][0:65, :],
                                vsb[kb][:, lh * 65:lh * 65 + 65],
                                pts[z][:, e * 512:(e + 1) * 512],
                                start=(kb == 0), stop=(kb == last_kb))

                pend = deque()
                for kp in range(npair):
                    kb0 = 2 * kp
                    diag = kp - (npair - 2)
                    pts = []
                    for z in range(2):
                        r0 = 64 * z
                        ps_ = psp.tile([128, 1024], F32, tag="ps", name="ps_")
                        pt = pt_ring[pt_ctr[0] % 6]
                        pt_ctr[0] += 1
                        for e in range(2):
                            kb = kb0 + e
                            m0 = 128 * (2 * diag + e) if diag >= 0 else 0
                            nc.tensor.matmul(
                                ps_[:, e * 512 + m0:(e + 1) * 512],
                                krot[hp][r0:r0 + 64, kb * 128:(kb + 1) * 128],
                                qrot[hp][r0:r0 + 64, c0 + m0:c0 + QC],
                                start=True, stop=True, tile_position=(r0, 0))
                        if diag < 0:
                            nc.scalar.activation(pt[:], ps_[:], Exp,
                                                 scale=0.125)
                        else:
                            for e in range(2):
                                m0 = 128 * (2 * diag + e)
                                nc.scalar.activation(
                                    pt[:, e * 512 + m0:(e + 1) * 512],
                                    ps_[:, e * 512 + m0:(e + 1) * 512],
                                    Exp, scale=0.125)
                            # full-tile mask: tri on diag blocks, zeros on
                            # uncomputed regions, ones elsewhere
                            nc.vector.tensor_mul(pt[:], pt[:],
                                                 pm_sb[diag][:])
                        pts.append(pt)
                    pend.append((kp, pts))
                    if len(pend) > 2:
                        emit_pv(*pend.popleft())
                if qc == 0:
                    # safe pump point: scores emitted, po chain not yet open
                    pump(4)
                while pend:
                    emit_pv(*pend.popleft())
                norm_chain(hp, qc, po)
                if ypool is not None:
                    for st in range(4 * qc, 4 * qc + 4):
                        fillers.append(lambda st=st: y_unit(ypool, st))

        def y_unit(ypool, st):
            """output projection for one 128-row s tile; bf16 stage + DMA."""
            ysb = wrk.tile([128, D], BF16, tag="ysb", bufs=3, name="ysb")
            for mc in range(2):
                py = ypool.tile([128, 512], F32, tag="py", name="py")
                for hp in range(2):
                    nc.tensor.matmul(py[:],
                                     otn[hp][:, st * 128:(st + 1) * 128],
                                     wo[:, hp * D + mc * 512:
                                        hp * D + (mc + 1) * 512],
                                     start=(hp == 0), stop=(hp == 1))
                dst = ysb[:, mc * 512:(mc + 1) * 512]
                if mc == 0:
                    nc.scalar.copy(dst, py[:])
                else:
                    nc.vector.tensor_copy(dst, py[:])
            dmaq[st % 3].dma_start(y_d[st * 128:(st + 1) * 128, :], ysb[:])

        # ---- emission ----
        with ExitStack() as s1:
            pp1 = s1.enter_context(tc.tile_pool(name="pp1", bufs=2,
                                                space="PSUM"))
            # minimal prologue: only what attention(0) qc0 needs — the first
            # q/k projection chunk (cols 0:512) and v tiles 0..3. The rest is
            # boundary-pumped in dependency-need order while DMAs stream in.
            punits = [(0, 0, 0), (1, 0, 0)]
            prev = None
            for u in punits:
                psb = proj_mm(pp1, *u)
                if prev is not None:
                    rope_mm(pp1, *prev[0], prev[1])
                prev = (u, psb)
            rope_mm(pp1, *prev[0], prev[1])
            for st in range(4):
                v_unit(pp1, st)
            for sc in range(1, 4):
                fillers.append(lambda sc=sc: proj_unit(pp1, 0, 0, sc))
                fillers.append(lambda sc=sc: proj_unit(pp1, 1, 0, sc))
            for st in range(4, 16):
                fillers.append(lambda st=st: v_unit(pp1, st))
            for qk in range(2):
                for sc in range(4):
                    fillers.append(
                        lambda qk=qk, sc=sc: proj_unit(pp1, qk, 1, sc))
            attention(0, boundary_pump=6)
            while fillers:
                pump()
        with ExitStack() as s2:
            pyp = s2.enter_context(tc.tile_pool(name="pyp", bufs=2,
                                                space="PSUM"))
            attention(1, ypool=pyp, boundary_pump=5)
            while fillers:
                pump()
        if DEBUG2:
            for j in range(2):
                nc.sync.dma_start(otn2_d[j], otn[j][:])
                nc.sync.dma_start(qr2_d[j], qrot[j][:])
                nc.sync.dma_start(kr2_d[j], krot[j][:])
            for st in range(16):
                nc.gpsimd.dma_start(v2_d[st], vsb[st][:])
            for i in range(6):
                nc.scalar.dma_start(pt2_d[i], pt_ring[i][:])
            for z in range(2):
                nc.sync.dma_start(po2_d[z], posnap[z][:])
                nc.sync.dma_start(nr2_d[z, 0:1, :], rdsnap[z][:])
                nc.sync.dma_start(nr2_d[z, 1:2, :], pbsnap[z][:])
        if DEBUG:
            for j in range(2):
                nc.sync.dma_start(qr_d[j], qrot[j][:])
                nc.sync.dma_start(kr_d[j], krot[j][:])
                nc.sync.dma_start(otn_d[j], otn[j][:])
            for st in range(4):
                nc.gpsimd.dma_start(v_dbg[st], vsb[st][:])

    nc.compile()
    return nc


def _get_nc():
    global _NC_CACHE
    if _NC_CACHE is None:
        _NC_CACHE = _build()
    return _NC_CACHE


def _host_prep(x, Wq, Wk, Wv, Wo):
    """Build the 8 per-core input maps."""
    bf = ml_dtypes.bfloat16
    x = np.asarray(x, dtype=np.float32)
    Wq, Wk, Wv, Wo = (np.asarray(w, dtype=np.float32) for w in (Wq, Wk, Wv, Wo))

    # rope tables: rows r = z*64 + d, angle index (r % 64) // 2
    t = np.arange(32, dtype=np.float64)
    theta = 1.0 / (ROPE_BASE ** (2.0 * t / HD))
    ang = np.arange(S, dtype=np.float64)[:, None] * theta[None, :]  # [S, 32]
    c32 = np.cos(ang).T.astype(np.float32)        # [32, S]
    s32 = np.sin(ang).T.astype(np.float32)
    cexp = np.repeat(c32, 2, axis=0)              # [64, S] rows 2i,2i+1 = c_i
    sexp = np.repeat(s32, 2, axis=0)
    cos_t = np.ascontiguousarray(np.tile(cexp, (2, 1))).astype(bf)   # [128, S]
    sin_t = np.ascontiguousarray(np.tile(sexp, (2, 1))).astype(bf)

    # signed permutation: out[2i] = -p[2i+1], out[2i+1] = p[2i]
    P = np.zeros((128, 128), dtype=np.float32)
    i2 = np.arange(0, 128, 2)
    P[i2 + 1, i2] = -1.0
    P[i2, i2 + 1] = 1.0
    P = np.ascontiguousarray(P.astype(bf))

    # pair masks pm[j][p, e*512+u] = 1 iff p <= u - 128*(2j+e): covers the
    # per-kb triangle, zeroes the uncomputed (trimmed) score regions, ones
    # elsewhere.
    p_ = np.arange(128)[:, None]
    u_ = np.arange(512)[None, :]
    pm = np.zeros((2, 128, 1024), dtype=np.float32)
    for j in range(2):
        for e in range(2):
            pm[j][:, e * 512:(e + 1) * 512] = (p_ <= u_ - 128 * (2 * j + e))
    pm = np.ascontiguousarray(pm.astype(bf))

    per_b = []
    for b in range(B):
        xtb = np.ascontiguousarray(x[b].T).astype(bf)   # [D, S]
        per_b.append(np.ascontiguousarray(xtb.reshape(NDT, 128, S)))

    def qk_tiles(W, g):
        A = W[g * LD:(g + 1) * LD, :]                   # [256, 1024]
        A4 = A.reshape(2, 128, NDT, 128)                # [jt, c, dt, p]
        return np.ascontiguousarray(
            A4.transpose(0, 3, 2, 1).reshape(2, 128, 1024).astype(bf))

    per_g = []
    for g in range(TPG):
        wq_t = qk_tiles(Wq, g)
        wk_t = qk_tiles(Wk, g)
        V = Wv[g * LD:(g + 1) * LD, :]                  # [256, 1024]
        V3 = V.reshape(256, NDT, 128)                   # [c, dt, p]
        wv_t = np.ascontiguousarray(
            V3.transpose(2, 1, 0).reshape(128, 2048).astype(bf))
        Wo_g = Wo[:, g * LD:(g + 1) * LD]               # [1024, 256]
        wo_t = np.ascontiguousarray(Wo_g.T.reshape(2, 128, D).astype(bf))
        per_g.append((wq_t, wk_t, wv_t, wo_t))

    in_maps = []
    for c in range(NCORES):
        b, g = divmod(c, TPG)
        wq_t, wk_t, wv_t, wo_t = per_g[g]
        in_maps.append({
            "xt": per_b[b],
            "wq": wq_t, "wk": wk_t, "wv": wv_t, "wo": wo_t,
            "cos": cos_t, "sin": sin_t, "perm": P, "pm": pm,
        })
    return in_maps


def run(inputs, trace=False):
    """Run on all 8 cores; returns (y_full, BassKernelResults)."""
    in_maps = _host_prep(inputs["x"], inputs["Wq"], inputs["Wk"],
                         inputs["Wv"], inputs["Wo"])
    nc = _get_nc()
    kw = {}
    if trace:
        kw = dict(trace=True, trace_cores=[0])
    res = run_bass_kernel_spmd(nc, in_maps, core_ids=list(range(NCORES)), **kw)
    y = np.zeros((B, S, D), dtype=np.float32)
    for c in range(NCORES):
        y[c // TPG] += np.asarray(res.results[c]["y"], dtype=np.float32)
    return y, res


def kernel(x, Wq, Wk, Wv, Wo, n_heads):
    assert int(n_heads) == H
    y, _ = run({"x": x, "Wq": Wq, "Wk": Wk, "Wv": Wv, "Wo": Wo})
    return y
